# revision 12
# baseline (speedup 1.0000x reference)
"""DND-LSTM cell (retrieval kNN + LSTM gates) on 8 Trainium2 NeuronCores.

Strategy: shard keys/vals along dict_len (L=100000), 12500/core (zero-padded
to 12544). Keys are normalized, scaled by 16 and cast to fp8e4 on the host, so
the device streams 6.5MB/core instead of 25.6MB (memory-bound regime) and
needs no on-device norm chain at all. Queries are normalized+scaled+fp8 on the
host too. Flash-softmax with the constant shift "-1" (cosine <= 1):

  ex[l, b]   = exp(sims[l, b] - 1)          sims = (16 kn_l) . (16 qn_b) / 256
  num[b, :]  += ex[l, b] * vals[l, :]        (fp8 vals, f32 PSUM accumulate)
  den[b]     += ex[l, b]                     (via an all-ones vals column)

All heavy matmuls run in fp8 DoubleRow mode (2 MACs/cell/cycle, HW-measured
110ns per sims tile and 158ns per av pair vs 230/222 without):

  PE   sm[128l, 256b] = DR-matmul(kt8[:, :, tile], qn8)     one MM per tile
  ACT  ex = exp(sm / 256 - 1) -> fp8, batched 6 l-tiles per ACTIVATE
       (the ~450ns per-instruction overhead amortizes)
  PE   av[bh][ck] += DR-matmul(ex pair, vt pair chunk)      129-col chunks
       (DoubleRow moving operands are capped at 2x256 free, so the 258 val
       columns split as [h0-127|den] and [h128-255|pad]; vals are laid out
       in that column order on the host)

The av matmuls are emitted AV_DELAY groups behind their sims group: the PE
queue is strict FIFO, so without the delay the PE idles ~1.3us per group
waiting for the ACT exp it needs before the av matmuls. Block sizes ladder up
so the first sims matmul only waits for a small first DMA. The zero-padded
tail rows contribute exactly 0 (vals rows incl. den column are 0), so no
ragged matmuls are needed.

LSTM gates are sharded over hidden dim (32 cols/core), bf16 weights,
sigmoid/tanh via the native Tanh spline so the whole kernel uses ONE ACT
table (exp_and_others: Exp + Tanh); the gate compute is emitted mid-stream
where its DMAs are long since complete. The host sums the 8 num/den partials
and applies the final elementwise combine.
"""

from collections import deque

import ml_dtypes
import numpy as np

import concourse.bacc as bacc
import concourse.mybir as mybir
import concourse.tile as tile
from concourse import bass_utils

F32 = mybir.dt.float32
BF16 = mybir.dt.bfloat16
F8 = mybir.dt.float8e4
AF = mybir.ActivationFunctionType
DR = mybir.MatmulPerfMode.DoubleRow

B = 256
D = 256
H = 256
NCORES = 8
HS = H // NCORES          # 32 hidden cols per core
GS = 5 * HS               # 160 gate cols per core
L = 100000
L_LOC = L // NCORES       # 12500 real keys per core
LPAD = ((L_LOC + 127) // 128) * 128  # 12544
NT_ALL = LPAD // 128      # 98 l-tiles
GRP = 6                   # l-tiles per batched exp (3 PSUM banks)
AVD = 2                   # groups of delay before av consumes ex
SCALE = 16.0              # host scale on kn/qn; exp scale = 1/SCALE^2
EPS = 1e-8
VW = H + 2                # vals row: h0-127 | den | h128-255 | pad

# tile counts per block: ladder up, then multiples of GRP, 2-tile tail
_BT = [6, 12, 24, 24, 24, 6, 2]
assert sum(_BT) == NT_ALL

_TABLES_PATCHED = False


def _patch_act_tables():
    """Resolve every ACT function to exp_and_others (has Exp AND Tanh), so
    the kernel performs exactly one ACT_TABLE_LOAD."""
    global _TABLES_PATCHED
    if _TABLES_PATCHED:
        return
    _TABLES_PATCHED = True
    orig = bacc.get_activation_tables

    def patched(arch):
        t = dict(orig(arch))
        keep = "exp_and_others"
        if keep in t:
            t = {name: (funcs if name == keep else set())
                 for name, funcs in t.items()}
        return t

    bacc.get_activation_tables = patched


def _blocks():
    out = []
    off = 0
    for nt in _BT:
        out.append((off, nt * 128))
        off += nt * 128
    return out


def _build():
    _patch_act_tables()
    nc = bacc.Bacc("TRN2", target_bir_lowering=False, debug=False,
                   num_devices=NCORES)

    k8 = nc.dram_tensor("k8", [128, 2 * LPAD], F8, kind="ExternalInput")
    v8 = nc.dram_tensor("v8", [128, NT_ALL * VW], F8, kind="ExternalInput")
    qn8 = nc.dram_tensor("qn8", [128, 2, B], F8, kind="ExternalInput")
    # hT | WhT | cT packed, partition rows 0:256
    p2 = nc.dram_tensor("p2", [256, B + GS + HS], BF16,
                        kind="ExternalInput")
    # xaT | WiT packed, partition rows 0:258
    p1 = nc.dram_tensor("p1", [D + 2, B + GS], BF16, kind="ExternalInput")

    nd = nc.dram_tensor("nd", [B, VW], F32, kind="ExternalOutput")
    org = nc.dram_tensor("org", [B, 3 * HS], F32, kind="ExternalOutput")

    W2 = B + GS + HS       # p2 row width
    W1 = B + GS            # p1 row width

    with tile.TileContext(nc) as tc:
        with (
            tc.tile_pool(name="const", bufs=1) as const,
            tc.tile_pool(name="sbA", bufs=2) as sbA,
            tc.tile_pool(name="kpool", bufs=3) as kpool,
            tc.tile_pool(name="vpool", bufs=3) as vpool,
            tc.tile_pool(name="smps", bufs=2, space="PSUM") as smps,
            tc.tile_pool(name="expool", bufs=4) as expool,
            tc.tile_pool(name="avps", bufs=1, space="PSUM") as avps,
        ):
            cm1 = const.tile([128, 1], F32)
            nc.vector.memset(cm1[:], -1.0)
            # DMA order is issue order on the Sync queue: qn8 (needed by the
            # first sims matmul) first, then the first two key/val blocks,
            # then the remaining small inputs.
            qt = const.tile([128, 2, B], F8, tag="qt", name="qt")
            nc.sync.dma_start(qt[:], qn8.ap())

            blts = _blocks()
            btiles = {}

            def emit_block_dma(bi):
                off, bs = blts[bi]
                nt = bs // 128
                kt = kpool.tile([128, 2, bs], F8, tag="kt", name="kt")
                nc.sync.dma_start(
                    kt[:], k8.ap()[:, 2 * off:2 * off + 2 * bs].rearrange(
                        "p (c l) -> p c l", c=2))
                vt = vpool.tile([128, nt, VW], F8, tag="vt", name="vt")
                t0 = off // 128
                nc.gpsimd.dma_start(
                    vt[:], v8.ap()[:, t0 * VW:(t0 + nt) * VW]
                    .rearrange("p (t h) -> p t h", t=nt))
                btiles[bi] = (kt, vt)

            emit_block_dma(0)
            emit_block_dma(1)

            sm2 = const.tile([128, 2, W2], BF16, tag="sm2", name="sm2")
            nc.gpsimd.dma_start(
                sm2[:], p2.ap().rearrange("(c p) w -> p c w", p=128))
            sm1 = const.tile([128, 2, W1], BF16, tag="sm1", name="sm1")
            nc.gpsimd.dma_start(
                sm1[:], p1.ap()[0:256, :].rearrange("(c p) w -> p c w", p=128))
            sm1b = const.tile([2, W1], BF16, tag="sm1b", name="sm1b")
            nc.gpsimd.dma_start(sm1b[:], p1.ap()[256:258, :])

            ha = [sm2[:, i, 0:B] for i in range(2)]
            wh = [sm2[:, i, B:B + GS] for i in range(2)]
            ctile = [sm2[:, i, B + GS:B + GS + HS] for i in range(2)]
            xa = [sm1[:, i, 0:B] for i in range(2)]
            wi = [sm1[:, i, B:B + GS] for i in range(2)]
            xa2 = sm1b[:, 0:B]
            wi2 = sm1b[:, B:B + GS]

            # av accumulators: one [128, 258] accumulator per bh. PSUM pool
            # tiles are bank-aligned, so bh0's bank-sized tile also hosts the
            # LSTM `pre` scratch in its spare columns (disjoint byte ranges).
            avA = avps.tile([128, 512], F32, tag="avA", name="avA")
            avB = avps.tile([128, VW], F32, tag="avB", name="avB")
            avsl = [avA[:, 0:VW], avB[:]]
            pre_t = avA[:, VW:VW + GS]

            def emit_lstm():
                for bh in range(2):
                    bsl = slice(bh * 128, (bh + 1) * 128)
                    pre = pre_t
                    nc.tensor.matmul(pre[:], xa[0][:, bsl], wi[0],
                                     start=True, stop=False)
                    nc.tensor.matmul(pre[:], xa[1][:, bsl], wi[1],
                                     start=False, stop=False)
                    nc.tensor.matmul(pre[:], xa2[:, bsl], wi2,
                                     start=False, stop=False)
                    nc.tensor.matmul(pre[:], ha[0][:, bsl], wh[0],
                                     start=False, stop=False)
                    nc.tensor.matmul(pre[:], ha[1][:, bsl], wh[1],
                                     start=False, stop=True)
                    # sigmoid(x) = 0.5*(1 + tanh(x/2)); tanh is in
                    # exp_and_others so no table switch
                    th = sbA.tile([128, 128], F32, tag="th")
                    nc.scalar.activation(th[:], pre[:, 0:128], AF.Tanh,
                                         scale=0.5)
                    gates = sbA.tile([128, GS], F32, tag="gates")
                    nc.vector.tensor_scalar(
                        gates[:, 0:128], th[:], 0.5, 0.5,
                        op0=mybir.AluOpType.mult, op1=mybir.AluOpType.add)
                    nc.scalar.activation(gates[:, 128:160], pre[:, 128:160],
                                         AF.Tanh)
                    # c_part = f*c + i*c~
                    fc = sbA.tile([128, HS], F32, tag="fc")
                    nc.vector.tensor_mul(fc[:], gates[:, 0:HS], ctile[bh])
                    ic = sbA.tile([128, HS], F32, tag="ic")
                    nc.vector.tensor_mul(ic[:], gates[:, HS:2 * HS],
                                         gates[:, 128:160])
                    cp = sbA.tile([128, HS], F32, tag="cp")
                    nc.vector.tensor_add(cp[:], fc[:], ic[:])
                    nc.sync.dma_start(org.ap()[bsl, 0:2 * HS],
                                      gates[:, 2 * HS:4 * HS])      # o | r
                    nc.sync.dma_start(org.ap()[bsl, 2 * HS:3 * HS], cp[:])

            # --- streamed kNN retrieval, DR matmuls, software-pipelined ---
            pend = deque()
            state = {"pair": 0}
            npair = NT_ALL // 2

            def emit_av(item):
                ex, vt, g0, ng, p0 = item
                for p in range(ng // 2):
                    first = p0 + p == 0
                    last = p0 + p == npair - 1
                    for bh in range(2):
                        nc.tensor.matmul(
                            avsl[bh],
                            ex[:, 2 * p:2 * p + 2, bh * 128:(bh + 1) * 128],
                            vt[:, g0 + 2 * p:g0 + 2 * p + 2, :],
                            start=first, stop=last, perf_mode=DR)

            for bi, (off, bs) in enumerate(blts):
                nt = bs // 128
                if bi + 1 < len(blts) and bi + 1 not in btiles:
                    emit_block_dma(bi + 1)
                kt, vt = btiles.pop(bi)
                for g0 in range(0, nt, GRP):
                    ng = min(GRP, nt - g0)
                    sm = smps.tile([128, GRP, B], F32, tag="sm", name="sm")
                    for j in range(ng):
                        lsl = slice((g0 + j) * 128, (g0 + j + 1) * 128)
                        nc.tensor.matmul(sm[:, j, :], kt[:, :, lsl], qt[:],
                                         start=True, stop=True, perf_mode=DR)
                    ex = expool.tile([128, GRP, B], F8, tag="ex", name="ex")
                    nc.scalar.activation(ex[:, 0:ng, :], sm[:, 0:ng, :],
                                         AF.Exp, bias=cm1[:],
                                         scale=1.0 / (SCALE * SCALE))
                    pend.append((ex, vt, g0, ng, state["pair"]))
                    state["pair"] += ng // 2
                    if len(pend) > AVD:
                        emit_av(pend.popleft())
                if bi == 2:
                    emit_lstm()
            while pend:
                emit_av(pend.popleft())

            for bh in range(2):
                avs = sbA.tile([128, VW], F32, tag="avs")
                nc.vector.tensor_copy(avs[:], avsl[bh])
                nc.sync.dma_start(nd.ap()[bh * 128:(bh + 1) * 128, :],
                                  avs[:])

    nc.compile()
    return nc


_NC_CACHE = {}


def _get_nc():
    if "nc" not in _NC_CACHE:
        _NC_CACHE["nc"] = _build()
    return _NC_CACHE["nc"]


def _shard_inputs(x_t, h, c, W_i2h, b_i2h, W_h2h, b_h2h, keys, vals):
    f = np.float32
    fp8 = ml_dtypes.float8_e4m3
    bf16 = ml_dtypes.bfloat16
    x_t = np.asarray(x_t, f)
    h = np.asarray(h, f)
    c = np.asarray(c, f)
    W_i2h = np.asarray(W_i2h, f)
    b_i2h = np.asarray(b_i2h, f)
    W_h2h = np.asarray(W_h2h, f)
    b_h2h = np.asarray(b_h2h, f)
    keys = np.asarray(keys, f)
    vals = np.asarray(vals, f)

    # host normalization (matches reference: x / max(||x||, eps))
    qn = x_t / np.maximum(np.linalg.norm(x_t, axis=1, keepdims=True), EPS)
    qn8 = np.ascontiguousarray(
        (SCALE * qn).T.reshape(2, 128, B).transpose(1, 0, 2)).astype(fp8)
    kn = keys / np.maximum(np.linalg.norm(keys, axis=1, keepdims=True), EPS)
    kn8 = (SCALE * kn).astype(fp8)           # [L, D] fp8
    v8f = vals.astype(fp8)                   # [L, H] fp8

    xaT = np.concatenate([x_t.T, np.ones((2, B), f)], axis=0).astype(bf16)
    hTb = h.T.astype(bf16)
    WiT_full = W_i2h.T  # [D, G]
    WhT_full = W_h2h.T  # [H, G]

    in_maps = []
    for k in range(NCORES):
        sl = slice(k * L_LOC, (k + 1) * L_LOC)
        kpad = np.zeros((LPAD, D), fp8)
        kpad[:L_LOC] = kn8[sl]
        parts = []
        for off, bs in _blocks():
            blkT = kpad[off:off + bs].T          # [256, bs]
            parts.append(blkT.reshape(2, 128, bs).transpose(1, 0, 2)
                         .reshape(128, 2 * bs))
        k8a = np.ascontiguousarray(np.concatenate(parts, axis=1))

        # vals row layout: h0-127 | den | h128-255 | pad (129-col DR chunks)
        vpad = np.zeros((LPAD, VW), fp8)
        vpad[:L_LOC, 0:128] = v8f[sl][:, 0:128]
        vpad[:L_LOC, 128] = fp8(1.0)  # denominator; pad rows stay 0
        vpad[:L_LOC, 129:257] = v8f[sl][:, 128:256]
        v8a = np.ascontiguousarray(
            vpad.reshape(NT_ALL, 128, VW).transpose(1, 0, 2)
            .reshape(128, NT_ALL * VW))

        gcols = np.concatenate(
            [np.arange(j * H + k * HS, j * H + (k + 1) * HS)
             for j in range(5)])
        # p2 = hT | WhT | c-as-bf16 (c rows are batch index; the DMA just
        # moves rows: c rows 0:128 -> chunk 0, 128:256 -> chunk 1)
        p2 = np.concatenate(
            [hTb, WhT_full[:, gcols].astype(bf16),
             np.ascontiguousarray(
                 c[:, k * HS:(k + 1) * HS]).astype(bf16).reshape(256, HS)],
            axis=1)
        p1 = np.concatenate(
            [xaT,
             np.concatenate([WiT_full[:, gcols], b_i2h[gcols][None, :],
                             b_h2h[gcols][None, :]], axis=0).astype(bf16)],
            axis=1)
        in_maps.append({
            "k8": k8a,
            "v8": v8a,
            "qn8": qn8,
            "p2": np.ascontiguousarray(p2),
            "p1": np.ascontiguousarray(p1.astype(bf16)),
        })
    return in_maps


def kernel(x_t, h, c, W_i2h, b_i2h, W_h2h, b_h2h, keys, vals):
    nc = _get_nc()
    in_maps = _shard_inputs(x_t, h, c, W_i2h, b_i2h, W_h2h, b_h2h, keys, vals)
    res = bass_utils.run_bass_kernel_spmd(
        nc, in_maps, core_ids=list(range(NCORES)))

    num = np.zeros((B, H), np.float64)
    den = np.zeros((B,), np.float64)
    for k in range(NCORES):
        ndk = res.results[k]["nd"]
        num[:, 0:128] += ndk[:, 0:128]
        num[:, 128:256] += ndk[:, 129:257]
        den += ndk[:, 128]
    m = np.tanh(num / den[:, None]).astype(np.float32)

    h_t = np.empty((B, H), np.float32)
    c_t = np.empty((B, H), np.float32)
    for k in range(NCORES):
        orgk = res.results[k]["org"]
        o = orgk[:, 0:HS]
        r = orgk[:, HS:2 * HS]
        cp = orgk[:, 2 * HS:3 * HS]
        hs = slice(k * HS, (k + 1) * HS)
        ct = cp + r * m[:, hs]
        c_t[:, hs] = ct
        h_t[:, hs] = o * np.tanh(ct)
    return (h_t, c_t)


# revision 13
# speedup vs baseline: 1.0247x; 1.0247x over previous
"""DND-LSTM cell (retrieval kNN + LSTM gates) on 8 Trainium2 NeuronCores.

Strategy: shard keys/vals along dict_len (L=100000), 12500/core (zero-padded
to 12544). Keys are normalized, scaled by 16 and cast to fp8e4 on the host, so
the device streams 6.5MB/core instead of 25.6MB (memory-bound regime) and
needs no on-device norm chain at all. Queries are normalized+scaled+fp8 on the
host too. Flash-softmax with the constant shift "-1" (cosine <= 1):

  ex[l, b]   = exp(sims[l, b] - 1)          sims = (16 kn_l) . (16 qn_b) / 256
  num[b, :]  += ex[l, b] * vals[l, :]        (fp8 vals, f32 PSUM accumulate)
  den[b]     += ex[l, b]                     (via an all-ones vals column)

All heavy matmuls run in fp8 DoubleRow mode (2 MACs/cell/cycle, HW-measured
110ns per sims tile and 158ns per av pair vs 230/222 without):

  PE   sm[128l, 256b] = DR-matmul(kt8[:, :, tile], qn8)     one MM per tile
  ACT  ex = exp(sm / 256 - 1) -> fp8, batched 6 l-tiles per ACTIVATE
       (the ~450ns per-instruction overhead amortizes)
  PE   av[bh][ck] += DR-matmul(ex pair, vt pair chunk)      129-col chunks
       (DoubleRow moving operands are capped at 2x256 free, so the 258 val
       columns split as [h0-127|den] and [h128-255|pad]; vals are laid out
       in that column order on the host)

The av matmuls are emitted AV_DELAY groups behind their sims group: the PE
queue is strict FIFO, so without the delay the PE idles ~1.3us per group
waiting for the ACT exp it needs before the av matmuls. Block sizes ladder up
so the first sims matmul only waits for a small first DMA. The zero-padded
tail rows contribute exactly 0 (vals rows incl. den column are 0), so no
ragged matmuls are needed.

LSTM gates are sharded over hidden dim (32 cols/core), bf16 weights,
sigmoid/tanh via the native Tanh spline so the whole kernel uses ONE ACT
table (exp_and_others: Exp + Tanh); the gate compute is emitted mid-stream
where its DMAs are long since complete. The host sums the 8 num/den partials
and applies the final elementwise combine.
"""

from collections import deque

import ml_dtypes
import numpy as np

import concourse.bacc as bacc
import concourse.mybir as mybir
import concourse.tile as tile
from concourse import bass_utils

F32 = mybir.dt.float32
BF16 = mybir.dt.bfloat16
F8 = mybir.dt.float8e4
AF = mybir.ActivationFunctionType
DR = mybir.MatmulPerfMode.DoubleRow

B = 256
D = 256
H = 256
NCORES = 8
HS = H // NCORES          # 32 hidden cols per core
GS = 5 * HS               # 160 gate cols per core
L = 100000
L_LOC = L // NCORES       # 12500 real keys per core
LPAD = ((L_LOC + 127) // 128) * 128  # 12544
NT_ALL = LPAD // 128      # 98 l-tiles
GRP = 6                   # l-tiles per batched exp (3 PSUM banks)
AVD = 2                   # groups of delay before av consumes ex
SCALE = 16.0              # host scale on kn/qn; exp scale = 1/SCALE^2
EPS = 1e-8
VW = H + 2                # vals row: h0-127 | den | h128-255 | pad

# tile counts per block: ladder up, then multiples of GRP, 2-tile tail
_BT = [6, 12, 24, 24, 24, 6, 2]
assert sum(_BT) == NT_ALL

_TABLES_PATCHED = False


def _patch_act_tables():
    """Resolve every ACT function to exp_and_others (has Exp AND Tanh), so
    the kernel performs exactly one ACT_TABLE_LOAD."""
    global _TABLES_PATCHED
    if _TABLES_PATCHED:
        return
    _TABLES_PATCHED = True
    orig = bacc.get_activation_tables

    def patched(arch):
        t = dict(orig(arch))
        keep = "exp_and_others"
        if keep in t:
            t = {name: (funcs if name == keep else set())
                 for name, funcs in t.items()}
        return t

    bacc.get_activation_tables = patched


def _blocks():
    out = []
    off = 0
    for nt in _BT:
        out.append((off, nt * 128))
        off += nt * 128
    return out


def _build():
    _patch_act_tables()
    nc = bacc.Bacc("TRN2", target_bir_lowering=False, debug=False,
                   num_devices=NCORES)

    kv8 = nc.dram_tensor("kv8", [128, 2 * LPAD + NT_ALL * VW], F8,
                         kind="ExternalInput")
    qn8 = nc.dram_tensor("qn8", [128, 2, B], F8, kind="ExternalInput")
    # hT | WhT | cT packed, partition rows 0:256
    p2 = nc.dram_tensor("p2", [256, B + GS + HS], BF16,
                        kind="ExternalInput")
    # xaT | WiT packed, partition rows 0:258
    p1 = nc.dram_tensor("p1", [D + 2, B + GS], BF16, kind="ExternalInput")

    nd = nc.dram_tensor("nd", [B, VW], F32, kind="ExternalOutput")
    org = nc.dram_tensor("org", [B, 3 * HS], F32, kind="ExternalOutput")

    W2 = B + GS + HS       # p2 row width
    W1 = B + GS            # p1 row width

    with tile.TileContext(nc) as tc:
        with (
            tc.tile_pool(name="const", bufs=1) as const,
            tc.tile_pool(name="sbA", bufs=2) as sbA,
            tc.tile_pool(name="kpool", bufs=3) as kpool,
            tc.tile_pool(name="smps", bufs=2, space="PSUM") as smps,
            tc.tile_pool(name="expool", bufs=4) as expool,
            tc.tile_pool(name="avps", bufs=1, space="PSUM") as avps,
        ):
            cm1 = const.tile([128, 1], F32)
            nc.vector.memset(cm1[:], -1.0)
            # DMA order is issue order on the Sync queue: qn8 (needed by the
            # first sims matmul) first, then the first two key/val blocks,
            # then the remaining small inputs.
            qt = const.tile([128, 2, B], F8, tag="qt", name="qt")
            nc.sync.dma_start(qt[:], qn8.ap())

            blts = _blocks()
            btiles = {}

            def emit_block_dma(bi):
                off, bs = blts[bi]
                nt = bs // 128
                w = 2 * bs + nt * VW
                coff = 2 * off + (off // 128) * VW
                kv = kpool.tile([128, w], F8, tag="kt", name="kv")
                nc.sync.dma_start(kv[:], kv8.ap()[:, coff:coff + w])
                kt = kv[:, 0:2 * bs].rearrange("p (c l) -> p c l", c=2)
                vt = kv[:, 2 * bs:w].rearrange("p (t h) -> p t h", t=nt)
                btiles[bi] = (kt, vt)

            emit_block_dma(0)
            emit_block_dma(1)

            sm2 = const.tile([128, 2, W2], BF16, tag="sm2", name="sm2")
            nc.sync.dma_start(
                sm2[:], p2.ap().rearrange("(c p) w -> p c w", p=128))
            sm1 = const.tile([128, 2, W1], BF16, tag="sm1", name="sm1")
            nc.sync.dma_start(
                sm1[:], p1.ap()[0:256, :].rearrange("(c p) w -> p c w", p=128))
            sm1b = const.tile([2, W1], BF16, tag="sm1b", name="sm1b")
            nc.sync.dma_start(sm1b[:], p1.ap()[256:258, :])

            ha = [sm2[:, i, 0:B] for i in range(2)]
            wh = [sm2[:, i, B:B + GS] for i in range(2)]
            ctile = [sm2[:, i, B + GS:B + GS + HS] for i in range(2)]
            xa = [sm1[:, i, 0:B] for i in range(2)]
            wi = [sm1[:, i, B:B + GS] for i in range(2)]
            xa2 = sm1b[:, 0:B]
            wi2 = sm1b[:, B:B + GS]

            # av accumulators: one [128, 258] accumulator per bh. PSUM pool
            # tiles are bank-aligned, so bh0's bank-sized tile also hosts the
            # LSTM `pre` scratch in its spare columns (disjoint byte ranges).
            avA = avps.tile([128, 512], F32, tag="avA", name="avA")
            avB = avps.tile([128, VW], F32, tag="avB", name="avB")
            avsl = [avA[:, 0:VW], avB[:]]
            pre_t = avA[:, VW:VW + GS]

            def emit_lstm():
                for bh in range(2):
                    bsl = slice(bh * 128, (bh + 1) * 128)
                    pre = pre_t
                    nc.tensor.matmul(pre[:], xa[0][:, bsl], wi[0],
                                     start=True, stop=False)
                    nc.tensor.matmul(pre[:], xa[1][:, bsl], wi[1],
                                     start=False, stop=False)
                    nc.tensor.matmul(pre[:], xa2[:, bsl], wi2,
                                     start=False, stop=False)
                    nc.tensor.matmul(pre[:], ha[0][:, bsl], wh[0],
                                     start=False, stop=False)
                    nc.tensor.matmul(pre[:], ha[1][:, bsl], wh[1],
                                     start=False, stop=True)
                    # sigmoid(x) = 0.5*(1 + tanh(x/2)); tanh is in
                    # exp_and_others so no table switch
                    th = sbA.tile([128, 128], F32, tag="th")
                    nc.scalar.activation(th[:], pre[:, 0:128], AF.Tanh,
                                         scale=0.5)
                    gates = sbA.tile([128, GS], F32, tag="gates")
                    nc.vector.tensor_scalar(
                        gates[:, 0:128], th[:], 0.5, 0.5,
                        op0=mybir.AluOpType.mult, op1=mybir.AluOpType.add)
                    nc.scalar.activation(gates[:, 128:160], pre[:, 128:160],
                                         AF.Tanh)
                    # c_part = f*c + i*c~
                    fc = sbA.tile([128, HS], F32, tag="fc")
                    nc.vector.tensor_mul(fc[:], gates[:, 0:HS], ctile[bh])
                    ic = sbA.tile([128, HS], F32, tag="ic")
                    nc.vector.tensor_mul(ic[:], gates[:, HS:2 * HS],
                                         gates[:, 128:160])
                    cp = sbA.tile([128, HS], F32, tag="cp")
                    nc.vector.tensor_add(cp[:], fc[:], ic[:])
                    nc.sync.dma_start(org.ap()[bsl, 0:2 * HS],
                                      gates[:, 2 * HS:4 * HS])      # o | r
                    nc.sync.dma_start(org.ap()[bsl, 2 * HS:3 * HS], cp[:])

            # --- streamed kNN retrieval, DR matmuls, software-pipelined ---
            pend = deque()
            state = {"pair": 0}
            npair = NT_ALL // 2

            def emit_av(item):
                ex, vt, g0, ng, p0 = item
                for p in range(ng // 2):
                    first = p0 + p == 0
                    last = p0 + p == npair - 1
                    for bh in range(2):
                        nc.tensor.matmul(
                            avsl[bh],
                            ex[:, 2 * p:2 * p + 2, bh * 128:(bh + 1) * 128],
                            vt[:, g0 + 2 * p:g0 + 2 * p + 2, :],
                            start=first, stop=last, perf_mode=DR)

            for bi, (off, bs) in enumerate(blts):
                nt = bs // 128
                if bi + 1 < len(blts) and bi + 1 not in btiles:
                    emit_block_dma(bi + 1)
                kt, vt = btiles.pop(bi)
                for g0 in range(0, nt, GRP):
                    ng = min(GRP, nt - g0)
                    sm = smps.tile([128, GRP, B], F32, tag="sm", name="sm")
                    for j in range(ng):
                        lsl = slice((g0 + j) * 128, (g0 + j + 1) * 128)
                        nc.tensor.matmul(sm[:, j, :], kt[:, :, lsl], qt[:],
                                         start=True, stop=True, perf_mode=DR)
                    ex = expool.tile([128, GRP, B], F8, tag="ex", name="ex")
                    nc.scalar.activation(ex[:, 0:ng, :], sm[:, 0:ng, :],
                                         AF.Exp, bias=cm1[:],
                                         scale=1.0 / (SCALE * SCALE))
                    pend.append((ex, vt, g0, ng, state["pair"]))
                    state["pair"] += ng // 2
                    if len(pend) > AVD:
                        emit_av(pend.popleft())
                if bi == 2:
                    emit_lstm()
            while pend:
                emit_av(pend.popleft())

            for bh in range(2):
                avs = sbA.tile([128, VW], F32, tag="avs")
                nc.vector.tensor_copy(avs[:], avsl[bh])
                nc.sync.dma_start(nd.ap()[bh * 128:(bh + 1) * 128, :],
                                  avs[:])

    nc.compile()
    return nc


_NC_CACHE = {}


def _get_nc():
    if "nc" not in _NC_CACHE:
        _NC_CACHE["nc"] = _build()
    return _NC_CACHE["nc"]


def _shard_inputs(x_t, h, c, W_i2h, b_i2h, W_h2h, b_h2h, keys, vals):
    f = np.float32
    fp8 = ml_dtypes.float8_e4m3
    bf16 = ml_dtypes.bfloat16
    x_t = np.asarray(x_t, f)
    h = np.asarray(h, f)
    c = np.asarray(c, f)
    W_i2h = np.asarray(W_i2h, f)
    b_i2h = np.asarray(b_i2h, f)
    W_h2h = np.asarray(W_h2h, f)
    b_h2h = np.asarray(b_h2h, f)
    keys = np.asarray(keys, f)
    vals = np.asarray(vals, f)

    # host normalization (matches reference: x / max(||x||, eps))
    qn = x_t / np.maximum(np.linalg.norm(x_t, axis=1, keepdims=True), EPS)
    qn8 = np.ascontiguousarray(
        (SCALE * qn).T.reshape(2, 128, B).transpose(1, 0, 2)).astype(fp8)
    kn = keys / np.maximum(np.linalg.norm(keys, axis=1, keepdims=True), EPS)
    kn8 = (SCALE * kn).astype(fp8)           # [L, D] fp8
    v8f = vals.astype(fp8)                   # [L, H] fp8

    xaT = np.concatenate([x_t.T, np.ones((2, B), f)], axis=0).astype(bf16)
    hTb = h.T.astype(bf16)
    WiT_full = W_i2h.T  # [D, G]
    WhT_full = W_h2h.T  # [H, G]

    in_maps = []
    for k in range(NCORES):
        sl = slice(k * L_LOC, (k + 1) * L_LOC)
        kpad = np.zeros((LPAD, D), fp8)
        kpad[:L_LOC] = kn8[sl]
        # vals row layout: h0-127 | den | h128-255 | pad
        vpad = np.zeros((LPAD, VW), fp8)
        vpad[:L_LOC, 0:128] = v8f[sl][:, 0:128]
        vpad[:L_LOC, 128] = fp8(1.0)  # denominator; pad rows stay 0
        vpad[:L_LOC, 129:257] = v8f[sl][:, 128:256]
        v8a = vpad.reshape(NT_ALL, 128, VW)
        # one contiguous [128, 2*bs + nt*VW] chunk per block: keysT then vals
        parts = []
        for off, bs in _blocks():
            nt = bs // 128
            blkT = kpad[off:off + bs].T          # [256, bs]
            parts.append(blkT.reshape(2, 128, bs).transpose(1, 0, 2)
                         .reshape(128, 2 * bs))
            t0 = off // 128
            parts.append(v8a[t0:t0 + nt].transpose(1, 0, 2)
                         .reshape(128, nt * VW))
        kv8a = np.ascontiguousarray(np.concatenate(parts, axis=1))

        gcols = np.concatenate(
            [np.arange(j * H + k * HS, j * H + (k + 1) * HS)
             for j in range(5)])
        # p2 = hT | WhT | c-as-bf16 (c rows are batch index; the DMA just
        # moves rows: c rows 0:128 -> chunk 0, 128:256 -> chunk 1)
        p2 = np.concatenate(
            [hTb, WhT_full[:, gcols].astype(bf16),
             np.ascontiguousarray(
                 c[:, k * HS:(k + 1) * HS]).astype(bf16).reshape(256, HS)],
            axis=1)
        p1 = np.concatenate(
            [xaT,
             np.concatenate([WiT_full[:, gcols], b_i2h[gcols][None, :],
                             b_h2h[gcols][None, :]], axis=0).astype(bf16)],
            axis=1)
        in_maps.append({
            "kv8": kv8a,
            "qn8": qn8,
            "p2": np.ascontiguousarray(p2),
            "p1": np.ascontiguousarray(p1.astype(bf16)),
        })
    return in_maps


def kernel(x_t, h, c, W_i2h, b_i2h, W_h2h, b_h2h, keys, vals):
    nc = _get_nc()
    in_maps = _shard_inputs(x_t, h, c, W_i2h, b_i2h, W_h2h, b_h2h, keys, vals)
    res = bass_utils.run_bass_kernel_spmd(
        nc, in_maps, core_ids=list(range(NCORES)))

    num = np.zeros((B, H), np.float64)
    den = np.zeros((B,), np.float64)
    for k in range(NCORES):
        ndk = res.results[k]["nd"]
        num[:, 0:128] += ndk[:, 0:128]
        num[:, 128:256] += ndk[:, 129:257]
        den += ndk[:, 128]
    m = np.tanh(num / den[:, None]).astype(np.float32)

    h_t = np.empty((B, H), np.float32)
    c_t = np.empty((B, H), np.float32)
    for k in range(NCORES):
        orgk = res.results[k]["org"]
        o = orgk[:, 0:HS]
        r = orgk[:, HS:2 * HS]
        cp = orgk[:, 2 * HS:3 * HS]
        hs = slice(k * HS, (k + 1) * HS)
        ct = cp + r * m[:, hs]
        c_t[:, hs] = ct
        h_t[:, hs] = o * np.tanh(ct)
    return (h_t, c_t)


# revision 15
# speedup vs baseline: 1.0595x; 1.0339x over previous
"""DND-LSTM cell (retrieval kNN + LSTM gates) on 8 Trainium2 NeuronCores.

Strategy: shard keys/vals along dict_len (L=100000), 12500/core (zero-padded
to 12544). Keys are normalized, scaled by 16 and cast to fp8e4 on the host, so
the device streams 6.5MB/core instead of 25.6MB (memory-bound regime) and
needs no on-device norm chain at all. Queries are normalized+scaled+fp8 on the
host too. Flash-softmax with the constant shift "-1" (cosine <= 1):

  ex[l, b]   = exp(sims[l, b] - 1)          sims = (16 kn_l) . (16 qn_b) / 256
  num[b, :]  += ex[l, b] * vals[l, :]        (fp8 vals, f32 PSUM accumulate)
  den[b]     += ex[l, b]                     (via an all-ones vals column)

All heavy matmuls run in fp8 DoubleRow mode (2 MACs/cell/cycle, HW-measured
110ns per sims tile and 158ns per av pair vs 230/222 without):

  PE   sm[128l, 256b] = DR-matmul(kt8[:, :, tile], qn8)     one MM per tile
  ACT  ex = exp(sm / 256 - 1) -> fp8, batched 6 l-tiles per ACTIVATE
       (the ~450ns per-instruction overhead amortizes)
  PE   av[bh][ck] += DR-matmul(ex pair, vt pair chunk)      129-col chunks
       (DoubleRow moving operands are capped at 2x256 free, so the 258 val
       columns split as [h0-127|den] and [h128-255|pad]; vals are laid out
       in that column order on the host)

The av matmuls are emitted AV_DELAY groups behind their sims group: the PE
queue is strict FIFO, so without the delay the PE idles ~1.3us per group
waiting for the ACT exp it needs before the av matmuls. Block sizes ladder up
so the first sims matmul only waits for a small first DMA. The zero-padded
tail rows contribute exactly 0 (vals rows incl. den column are 0), so no
ragged matmuls are needed.

LSTM gates are sharded over hidden dim (32 cols/core), bf16 weights,
sigmoid/tanh via the native Tanh spline so the whole kernel uses ONE ACT
table (exp_and_others: Exp + Tanh); the gate compute is emitted mid-stream
where its DMAs are long since complete. The host sums the 8 num/den partials
and applies the final elementwise combine.
"""

from collections import deque

import ml_dtypes
import numpy as np

import concourse.bacc as bacc
import concourse.mybir as mybir
import concourse.tile as tile
from concourse import bass_utils

F32 = mybir.dt.float32
BF16 = mybir.dt.bfloat16
F8 = mybir.dt.float8e4
AF = mybir.ActivationFunctionType
DR = mybir.MatmulPerfMode.DoubleRow

B = 256
D = 256
H = 256
NCORES = 8
HS = H // NCORES          # 32 hidden cols per core
GS = 5 * HS               # 160 gate cols per core
L = 100000
L_LOC = L // NCORES       # 12500 real keys per core
LPAD = ((L_LOC + 127) // 128) * 128  # 12544
NT_ALL = LPAD // 128      # 98 l-tiles
GRP = 6                   # l-tiles per batched exp (3 PSUM banks)
AVD = 3                   # groups of delay before av consumes ex
SCALE = 16.0              # host scale on kn/qn; exp scale = 1/SCALE^2
EPS = 1e-8
VW = H + 2                # vals row: h0-127 | den | h128-255 | pad

# tile counts per block: ladder up, then multiples of GRP, 2-tile tail
_BT = [6, 12, 24, 24, 24, 6, 2]
assert sum(_BT) == NT_ALL

_TABLES_PATCHED = False


def _patch_act_tables():
    """Resolve every ACT function to exp_and_others (has Exp AND Tanh), so
    the kernel performs exactly one ACT_TABLE_LOAD."""
    global _TABLES_PATCHED
    if _TABLES_PATCHED:
        return
    _TABLES_PATCHED = True
    orig = bacc.get_activation_tables

    def patched(arch):
        t = dict(orig(arch))
        keep = "exp_and_others"
        if keep in t:
            t = {name: (funcs if name == keep else set())
                 for name, funcs in t.items()}
        return t

    bacc.get_activation_tables = patched


def _blocks():
    out = []
    off = 0
    for nt in _BT:
        out.append((off, nt * 128))
        off += nt * 128
    return out


def _build():
    _patch_act_tables()
    nc = bacc.Bacc("TRN2", target_bir_lowering=False, debug=False,
                   num_devices=NCORES)

    # block 0's chunk carries the fp8 query tile at its head (512B/partition)
    kv8 = nc.dram_tensor("kv8", [128, 512 + 2 * LPAD + NT_ALL * VW], F8,
                         kind="ExternalInput")
    # hT | WhT | cT packed, partition rows 0:256
    p2 = nc.dram_tensor("p2", [256, B + GS + HS], BF16,
                        kind="ExternalInput")
    # xaT | WiT packed, partition rows 0:258
    p1 = nc.dram_tensor("p1", [D + 2, B + GS], BF16, kind="ExternalInput")

    nd = nc.dram_tensor("nd", [B, VW], F32, kind="ExternalOutput")
    org = nc.dram_tensor("org", [B, 3 * HS], F32, kind="ExternalOutput")

    W2 = B + GS + HS       # p2 row width
    W1 = B + GS            # p1 row width

    with tile.TileContext(nc) as tc:
        with (
            tc.tile_pool(name="const", bufs=1) as const,
            tc.tile_pool(name="sbA", bufs=2) as sbA,
            tc.tile_pool(name="kv0p", bufs=1) as kv0p,
            tc.tile_pool(name="kpool", bufs=3) as kpool,
            tc.tile_pool(name="smps", bufs=2, space="PSUM") as smps,
            tc.tile_pool(name="expool", bufs=5) as expool,
            tc.tile_pool(name="avps", bufs=1, space="PSUM") as avps,
        ):
            cm1 = const.tile([128, 1], F32)
            nc.vector.memset(cm1[:], -1.0)
            blts = _blocks()
            btiles = {}

            def emit_block_dma(bi):
                off, bs = blts[bi]
                nt = bs // 128
                q = 512 if bi == 0 else 0
                w = q + 2 * bs + nt * VW
                coff = 512 + 2 * off + (off // 128) * VW - q
                pool = kv0p if bi == 0 else kpool
                kv = pool.tile([128, w], F8, tag="kt", name="kv")
                nc.sync.dma_start(kv[:], kv8.ap()[:, coff:coff + w])
                kt = kv[:, q:q + 2 * bs].rearrange("p (c l) -> p c l", c=2)
                vt = kv[:, q + 2 * bs:w].rearrange("p (t h) -> p t h", t=nt)
                btiles[bi] = (kt, vt)
                return kv

            # block 0 stays resident all kernel (own pool): the query tile
            # at its head is read by every sims matmul, no copy needed
            kv0 = emit_block_dma(0)
            qt = kv0[:, 0:512].rearrange("p (c b) -> p c b", c=2)
            emit_block_dma(1)
            emit_block_dma(2)

            sm2 = const.tile([128, 2, W2], BF16, tag="sm2", name="sm2")
            nc.sync.dma_start(
                sm2[:], p2.ap().rearrange("(c p) w -> p c w", p=128))
            sm1 = const.tile([128, 2, W1], BF16, tag="sm1", name="sm1")
            nc.sync.dma_start(
                sm1[:], p1.ap()[0:256, :].rearrange("(c p) w -> p c w", p=128))
            sm1b = const.tile([2, W1], BF16, tag="sm1b", name="sm1b")
            nc.sync.dma_start(sm1b[:], p1.ap()[256:258, :])

            ha = [sm2[:, i, 0:B] for i in range(2)]
            wh = [sm2[:, i, B:B + GS] for i in range(2)]
            ctile = [sm2[:, i, B + GS:B + GS + HS] for i in range(2)]
            xa = [sm1[:, i, 0:B] for i in range(2)]
            wi = [sm1[:, i, B:B + GS] for i in range(2)]
            xa2 = sm1b[:, 0:B]
            wi2 = sm1b[:, B:B + GS]

            # av accumulators: one [128, 258] accumulator per bh. PSUM pool
            # tiles are bank-aligned, so bh0's bank-sized tile also hosts the
            # LSTM `pre` scratch in its spare columns (disjoint byte ranges).
            avA = avps.tile([128, 512], F32, tag="avA", name="avA")
            avB = avps.tile([128, VW], F32, tag="avB", name="avB")
            avsl = [avA[:, 0:VW], avB[:]]
            pre_t = avA[:, VW:VW + GS]

            def emit_lstm():
                for bh in range(2):
                    bsl = slice(bh * 128, (bh + 1) * 128)
                    pre = pre_t
                    nc.tensor.matmul(pre[:], xa[0][:, bsl], wi[0],
                                     start=True, stop=False)
                    nc.tensor.matmul(pre[:], xa[1][:, bsl], wi[1],
                                     start=False, stop=False)
                    nc.tensor.matmul(pre[:], xa2[:, bsl], wi2,
                                     start=False, stop=False)
                    nc.tensor.matmul(pre[:], ha[0][:, bsl], wh[0],
                                     start=False, stop=False)
                    nc.tensor.matmul(pre[:], ha[1][:, bsl], wh[1],
                                     start=False, stop=True)
                    # sigmoid(x) = 0.5*(1 + tanh(x/2)); tanh is in
                    # exp_and_others so no table switch
                    th = sbA.tile([128, 128], F32, tag="th")
                    nc.scalar.activation(th[:], pre[:, 0:128], AF.Tanh,
                                         scale=0.5)
                    gates = sbA.tile([128, GS], F32, tag="gates")
                    nc.vector.tensor_scalar(
                        gates[:, 0:128], th[:], 0.5, 0.5,
                        op0=mybir.AluOpType.mult, op1=mybir.AluOpType.add)
                    nc.scalar.activation(gates[:, 128:160], pre[:, 128:160],
                                         AF.Tanh)
                    # c_part = f*c + i*c~
                    fc = sbA.tile([128, HS], F32, tag="fc")
                    nc.vector.tensor_mul(fc[:], gates[:, 0:HS], ctile[bh])
                    ic = sbA.tile([128, HS], F32, tag="ic")
                    nc.vector.tensor_mul(ic[:], gates[:, HS:2 * HS],
                                         gates[:, 128:160])
                    cp = sbA.tile([128, HS], F32, tag="cp")
                    nc.vector.tensor_add(cp[:], fc[:], ic[:])
                    nc.sync.dma_start(org.ap()[bsl, 0:2 * HS],
                                      gates[:, 2 * HS:4 * HS])      # o | r
                    nc.sync.dma_start(org.ap()[bsl, 2 * HS:3 * HS], cp[:])

            # --- streamed kNN retrieval, DR matmuls, software-pipelined ---
            pend = deque()
            state = {"pair": 0}
            npair = NT_ALL // 2

            def emit_av(item):
                ex, vt, g0, ng, p0 = item
                for p in range(ng // 2):
                    first = p0 + p == 0
                    last = p0 + p == npair - 1
                    for bh in range(2):
                        nc.tensor.matmul(
                            avsl[bh],
                            ex[:, 2 * p:2 * p + 2, bh * 128:(bh + 1) * 128],
                            vt[:, g0 + 2 * p:g0 + 2 * p + 2, :],
                            start=first, stop=last, perf_mode=DR)

            for bi, (off, bs) in enumerate(blts):
                nt = bs // 128
                if bi + 2 < len(blts) and bi + 2 not in btiles:
                    emit_block_dma(bi + 2)
                kt, vt = btiles.pop(bi)
                for g0 in range(0, nt, GRP):
                    ng = min(GRP, nt - g0)
                    sm = smps.tile([128, GRP, B], F32, tag="sm", name="sm")
                    for j in range(ng):
                        lsl = slice((g0 + j) * 128, (g0 + j + 1) * 128)
                        nc.tensor.matmul(sm[:, j, :], kt[:, :, lsl], qt,
                                         start=True, stop=True, perf_mode=DR)
                    ex = expool.tile([128, GRP, B], F8, tag="ex", name="ex")
                    nc.scalar.activation(ex[:, 0:ng, :], sm[:, 0:ng, :],
                                         AF.Exp, bias=cm1[:],
                                         scale=1.0 / (SCALE * SCALE))
                    pend.append((ex, vt, g0, ng, state["pair"]))
                    state["pair"] += ng // 2
                    if len(pend) > AVD:
                        emit_av(pend.popleft())
                if bi == 2:
                    emit_lstm()
            while pend:
                emit_av(pend.popleft())

            for bh in range(2):
                avs = sbA.tile([128, VW], F32, tag="avs")
                nc.vector.tensor_copy(avs[:], avsl[bh])
                nc.sync.dma_start(nd.ap()[bh * 128:(bh + 1) * 128, :],
                                  avs[:])

    nc.compile()
    return nc


_NC_CACHE = {}


def _get_nc():
    if "nc" not in _NC_CACHE:
        _NC_CACHE["nc"] = _build()
    return _NC_CACHE["nc"]


def _shard_inputs(x_t, h, c, W_i2h, b_i2h, W_h2h, b_h2h, keys, vals):
    f = np.float32
    fp8 = ml_dtypes.float8_e4m3
    bf16 = ml_dtypes.bfloat16
    x_t = np.asarray(x_t, f)
    h = np.asarray(h, f)
    c = np.asarray(c, f)
    W_i2h = np.asarray(W_i2h, f)
    b_i2h = np.asarray(b_i2h, f)
    W_h2h = np.asarray(W_h2h, f)
    b_h2h = np.asarray(b_h2h, f)
    keys = np.asarray(keys, f)
    vals = np.asarray(vals, f)

    # host normalization (matches reference: x / max(||x||, eps))
    qn = x_t / np.maximum(np.linalg.norm(x_t, axis=1, keepdims=True), EPS)
    qn8 = np.ascontiguousarray(
        (SCALE * qn).T.reshape(2, 128, B).transpose(1, 0, 2)
        .reshape(128, 512)).astype(fp8)
    kn = keys / np.maximum(np.linalg.norm(keys, axis=1, keepdims=True), EPS)
    kn8 = (SCALE * kn).astype(fp8)           # [L, D] fp8
    v8f = vals.astype(fp8)                   # [L, H] fp8

    xaT = np.concatenate([x_t.T, np.ones((2, B), f)], axis=0).astype(bf16)
    hTb = h.T.astype(bf16)
    WiT_full = W_i2h.T  # [D, G]
    WhT_full = W_h2h.T  # [H, G]

    in_maps = []
    for k in range(NCORES):
        sl = slice(k * L_LOC, (k + 1) * L_LOC)
        kpad = np.zeros((LPAD, D), fp8)
        kpad[:L_LOC] = kn8[sl]
        # vals row layout: h0-127 | den | h128-255 | pad
        vpad = np.zeros((LPAD, VW), fp8)
        vpad[:L_LOC, 0:128] = v8f[sl][:, 0:128]
        vpad[:L_LOC, 128] = fp8(1.0)  # denominator; pad rows stay 0
        vpad[:L_LOC, 129:257] = v8f[sl][:, 128:256]
        v8a = vpad.reshape(NT_ALL, 128, VW)
        # one contiguous chunk per block: keysT then vals; block 0 leads
        # with the 512B/partition query tile
        parts = [qn8]
        for off, bs in _blocks():
            nt = bs // 128
            blkT = kpad[off:off + bs].T          # [256, bs]
            parts.append(blkT.reshape(2, 128, bs).transpose(1, 0, 2)
                         .reshape(128, 2 * bs))
            t0 = off // 128
            parts.append(v8a[t0:t0 + nt].transpose(1, 0, 2)
                         .reshape(128, nt * VW))
        kv8a = np.ascontiguousarray(np.concatenate(parts, axis=1))

        gcols = np.concatenate(
            [np.arange(j * H + k * HS, j * H + (k + 1) * HS)
             for j in range(5)])
        # p2 = hT | WhT | c-as-bf16 (c rows are batch index; the DMA just
        # moves rows: c rows 0:128 -> chunk 0, 128:256 -> chunk 1)
        p2 = np.concatenate(
            [hTb, WhT_full[:, gcols].astype(bf16),
             np.ascontiguousarray(
                 c[:, k * HS:(k + 1) * HS]).astype(bf16).reshape(256, HS)],
            axis=1)
        p1 = np.concatenate(
            [xaT,
             np.concatenate([WiT_full[:, gcols], b_i2h[gcols][None, :],
                             b_h2h[gcols][None, :]], axis=0).astype(bf16)],
            axis=1)
        in_maps.append({
            "kv8": kv8a,
            "p2": np.ascontiguousarray(p2),
            "p1": np.ascontiguousarray(p1.astype(bf16)),
        })
    return in_maps


def kernel(x_t, h, c, W_i2h, b_i2h, W_h2h, b_h2h, keys, vals):
    nc = _get_nc()
    in_maps = _shard_inputs(x_t, h, c, W_i2h, b_i2h, W_h2h, b_h2h, keys, vals)
    res = bass_utils.run_bass_kernel_spmd(
        nc, in_maps, core_ids=list(range(NCORES)))

    num = np.zeros((B, H), np.float64)
    den = np.zeros((B,), np.float64)
    for k in range(NCORES):
        ndk = res.results[k]["nd"]
        num[:, 0:128] += ndk[:, 0:128]
        num[:, 128:256] += ndk[:, 129:257]
        den += ndk[:, 128]
    m = np.tanh(num / den[:, None]).astype(np.float32)

    h_t = np.empty((B, H), np.float32)
    c_t = np.empty((B, H), np.float32)
    for k in range(NCORES):
        orgk = res.results[k]["org"]
        o = orgk[:, 0:HS]
        r = orgk[:, HS:2 * HS]
        cp = orgk[:, 2 * HS:3 * HS]
        hs = slice(k * HS, (k + 1) * HS)
        ct = cp + r * m[:, hs]
        c_t[:, hs] = ct
        h_t[:, hs] = o * np.tanh(ct)
    return (h_t, c_t)
